# revision 13
# baseline (speedup 1.0000x reference)
"""GNN message-passing (graph convolution) kernel for 8 Trainium2 NeuronCores.

    out = relu(segment_sum(h[col], row) + bias),  h = x @ W

Strategy (dst-block sharding -- no collectives needed):
  * Host sorts edges by destination node, buckets them into 157 blocks of 128
    dst nodes, sorts blocks by edge count and deals them snake-style into 20
    slots x 8 cores so every core runs the same instruction stream with
    near-balanced per-slot chunk budgets.  Each core produces a disjoint slice
    of the output, so no all-reduce is needed.
  * Phase A (per core, replicated): h = x @ W on the PE in fp16 (PSUM fp32
    accumulate).  x is shipped pre-transposed in a partition-major layout so
    the whole 10 MB streams in as 5 large DMAs (16 KB contiguous per
    partition per DMA -- full DMA-bus rate, HWDGE amortized).  h tiles are
    copied PSUM->SBUF on the DVE and stored to a DRAM buffer h[20096,128]
    fp16 in 16-tile batched DMAs.
  * Phase B: per dst slot, SWDGE dma_gather fetches the slot's (padded) edge
    h rows into SBUF [128e, K, 128f] in sub-gathers of 16 chunks (2048
    descriptors; the SWDGE ring is enlarged via dynamic_dma_scratch_size);
    the DVE builds one-hot tiles S[e,n] = (iota == rowloc) in fp16 (waiting
    on the gather so the PE needs only one wait per matmul); the PE
    accumulates out_block = ones^T @ bias + sum_c S_c^T @ val_c in PSUM fp32
    -- an exact segment-sum with the bias folded in as a K=1 matmul.  ACT
    applies ReLU PSUM->SBUF and results stream out per slot.

Numerics: fp16 operands with fp32 accumulation everywhere; the one-hot matmul
is exact, so the only error is fp16 rounding of x, W and h (~1e-3 relative).
"""

import sys

import numpy as np

sys.path.insert(0, "/opt/trn_rl_repo")

import concourse.bacc as bacc  # noqa: E402
import concourse.bass as bass  # noqa: E402  (engine types)
import concourse.mybir as mybir  # noqa: E402
from concourse.bass_utils import run_bass_kernel_spmd  # noqa: E402

N_NODES = 20000
FIN = 256
FOUT = 128
N_EDGES = 640000

NT = 157                 # node tiles of 128 (nodes padded to 20096)
NPAD = NT * 128          # 20096
NBLK = 157               # dst blocks of 128 nodes
NCORES = 8
NB = 20                  # block slots per core

XT_CH = 32               # xt tiles per load DMA (5 DMAs: 32*4 + 29)
HG = 16                  # h tiles per store DMA (10 DMAs: 16*9 + 13)
SUBG = 7                 # chunks per sub-gather (896 descriptors)
PSA = 4                  # phase-A psum ring
SB = 4                   # one-hot tile ring

FP16 = mybir.dt.float16
FP32 = mybir.dt.float32
I16 = mybir.dt.int16


def _chunks(total, step):
    out = []
    o = 0
    while o < total:
        out.append((o, min(step, total - o)))
        o += step
    return out


def _host_prep(x, edge_index, weight, bias):
    """Cast/retile operands, bucket + balance edges by destination block.

    Returns (shared, per_core, plan): shared tensors (same on every core),
    per-core col/rl tensors, and the static plan consumed by _build_program.
    """
    x = np.asarray(x, np.float32)
    weight = np.asarray(weight, np.float32)
    bias = np.asarray(bias, np.float32)

    xpad = np.zeros((NPAD, FIN), np.float32)
    xpad[:N_NODES] = x
    # partition-major lhsT: xt[k, i, kc, m] = x[i*128+m, kc*128+k]
    xt = np.ascontiguousarray(
        xpad.reshape(NT, 128, 2, 128).transpose(3, 0, 2, 1).astype(np.float16)
    ).reshape(128, NT * 2 * 128)
    w_t = np.ascontiguousarray(weight.astype(np.float16).reshape(2, 128, 128))
    ones16 = np.ones((1, 128), np.float16)
    bias16 = np.ascontiguousarray(bias.astype(np.float16).reshape(1, 128))
    iota16 = np.ascontiguousarray(
        np.broadcast_to(np.arange(128, dtype=np.float16), (128, 128))
    )

    row = np.asarray(edge_index[0]).astype(np.int64)
    col = np.asarray(edge_index[1]).astype(np.int64)
    order = np.argsort(row, kind="stable")
    rs = row[order].astype(np.int32)
    cs = col[order].astype(np.int32)

    blk = rs >> 7
    counts = np.bincount(blk, minlength=NBLK)
    starts = np.concatenate([[0], np.cumsum(counts)])

    # deal blocks (sorted by descending count) into slots: slot s holds the
    # 8 blocks ranked [8s, 8s+8); per-slot chunk budget = max count in group
    rank = np.argsort(-counts, kind="stable")
    block_of = [[None] * NB for _ in range(NCORES)]
    K = [0] * NB                      # chunks per slot
    for s in range(NB):
        grp = rank[8 * s:8 * s + 8]
        for c, b in enumerate(grp):
            block_of[c][s] = int(b)
        K[s] = max(1, -(-int(counts[grp].max()) // 128))
    KTOT = sum(K)
    subs = [_chunks(K[s], SUBG) for s in range(NB)]       # (chunk_off, len)
    GMAX = max(len(g) for g in subs)

    # per-core index/rowloc buffers
    cidx = 8 * KTOT                    # int16 per partition (idxs: L/16 * 8)
    col16 = np.zeros((NCORES, 128, cidx), np.int16)
    rl16 = np.full((NCORES, 128, KTOT), -1.0, np.float32)
    for core in range(NCORES):
        co = 0
        ko = 0
        for s in range(NB):
            n_pad = K[s] * 128
            lin_col = np.zeros(n_pad, np.int32)
            lin_rl = np.full(n_pad, -1.0, np.float32)
            b = block_of[core][s]
            if b is not None:
                e0, e1 = int(starts[b]), int(starts[b + 1])
                k = e1 - e0
                lin_col[:k] = cs[e0:e1]
                lin_rl[:k] = rs[e0:e1] - b * 128
            for g0, gl in subs[s]:
                L = gl * 128
                piece = lin_col[g0 * 128:g0 * 128 + L]
                col16[core, :, co:co + L // 16] = np.tile(
                    piece.reshape(L // 16, 16).T.astype(np.int16), (8, 1)
                )
                co += L // 16
            rl16[core, :, ko:ko + K[s]] = (
                lin_rl.reshape(K[s], 128).T.astype(np.float32)
            )
            ko += K[s]
        assert co == cidx and ko == KTOT

    shared = {
        "xt": xt,
        "w": w_t,
        "ones": ones16,
        "bias": bias16,
        "iota": iota16,
    }
    per_core = [
        {"col": np.ascontiguousarray(col16[c]),
         "rl": np.ascontiguousarray(rl16[c])}
        for c in range(NCORES)
    ]
    plan = {"K": K, "KTOT": KTOT, "subs": subs, "GMAX": GMAX, "cidx": cidx,
            "block_of": block_of}
    return shared, per_core, plan


def _build_program(plan):
    K, KTOT, subs, GMAX, cidx = (
        plan["K"], plan["KTOT"], plan["subs"], plan["GMAX"], plan["cidx"]
    )
    nc = bacc.Bacc("TRN2", dynamic_dma_scratch_size=16384)

    xt_d = nc.dram_tensor("xt", [128, NT * 2 * 128], FP16, kind="ExternalInput")
    w_d = nc.dram_tensor("w", [2, 128, 128], FP16, kind="ExternalInput")
    on_d = nc.dram_tensor("ones", [1, 128], FP16, kind="ExternalInput")
    b_d = nc.dram_tensor("bias", [1, 128], FP16, kind="ExternalInput")
    io_d = nc.dram_tensor("iota", [128, 128], FP16, kind="ExternalInput")
    col_d = nc.dram_tensor("col", [128, cidx], I16, kind="ExternalInput")
    rl_d = nc.dram_tensor("rl", [128, KTOT], FP32, kind="ExternalInput")
    h_d = nc.dram_tensor("hbuf", [NPAD, 128], FP16)
    o_d = nc.dram_tensor("out", [NB * 128, 128], FP32, kind="ExternalOutput")

    xt_chunks = _chunks(NT, XT_CH)     # (tile_off, n_tiles)
    h_groups = _chunks(NT, HG)
    NXT = len(xt_chunks)
    NHG = len(h_groups)
    chunk_of_tile = []
    for r, (t0, tn) in enumerate(xt_chunks):
        chunk_of_tile += [r] * tn

    KMAX = max(K)
    # retire count of the phase-B matmul that consumes global chunk j
    mm_retire = []
    mm_cum = [0]                       # matmuls through slot s (incl. bias)
    j = 0
    for s in range(NB):
        base = mm_cum[-1] + 1          # bias matmul of slot s
        for c in range(K[s]):
            mm_retire.append(base + c + 1)
        mm_cum.append(base + K[s])
        j += K[s]
    # sub-gather bookkeeping: for global chunk j -> (slot, sub index, first)
    # and the cumulative increment count of each s_gat sem at each use
    sub_of_chunk = []
    gat_count = {}                     # (parity, gi) -> uses so far
    gat_wait = {}                      # (s, gi) -> wait value (x16)
    for s in range(NB):
        for gi, (g0, gl) in enumerate(subs[s]):
            key = (s % 2, gi)
            gat_count[key] = gat_count.get(key, 0) + 1
            gat_wait[(s, gi)] = gat_count[key]
            sub_of_chunk += [(s, gi, g0)] * gl
    # idx (int16-per-partition) offset of each sub-gather, per slot
    sub_off = []
    co = 0
    for s in range(NB):
        offs = []
        for g0, gl in subs[s]:
            offs.append(co)
            co += gl * 8
        sub_off.append(offs)

    from contextlib import ExitStack

    with ExitStack() as es:
        ph = [es.enter_context(nc.psum_tensor(f"ph{k}", [128, 512], FP32))
              for k in range(PSA)]
        pbk = [es.enter_context(nc.psum_tensor(f"pb{k}", [128, 512], FP32))
               for k in range(2)]
        w_sb = es.enter_context(nc.sbuf_tensor("w_sb", [128, 2, 128], FP16))
        on_sb = es.enter_context(nc.sbuf_tensor("on_sb", [1, 128], FP16))
        b_sb = es.enter_context(nc.sbuf_tensor("b_sb", [1, 128], FP16))
        iota_sb = es.enter_context(nc.sbuf_tensor("iota_sb", [128, 128], FP16))
        col_sb = es.enter_context(nc.sbuf_tensor("col_sb", [128, cidx], I16))
        rl_sb = es.enter_context(nc.sbuf_tensor("rl_sb", [128, KTOT], FP32))
        xt_sb = es.enter_context(
            nc.sbuf_tensor("xt_sb", [128, 2, XT_CH, 2, 128], FP16)
        )
        h_sb = es.enter_context(nc.sbuf_tensor("h_sb", [128, 2 * HG, 128], FP16))
        val_sb = es.enter_context(
            nc.sbuf_tensor("val_sb", [128, 2, KMAX, 128], FP16)
        )
        s_sb = es.enter_context(nc.sbuf_tensor("s_sb", [128, SB, 128], FP16))
        o_sb = es.enter_context(nc.sbuf_tensor("o_sb", [128, 2, 128], FP32))

        s_ld = es.enter_context(nc.semaphore("s_ld"))       # one-time loads
        s_xt = [es.enter_context(nc.semaphore(f"s_xt{k}")) for k in range(2)]
        s_hw = [es.enter_context(nc.semaphore(f"s_hw{k}")) for k in range(2)]
        s_gat = [
            es.enter_context(nc.semaphore(f"s_gat{k}"))
            for k in range(2 * GMAX)
        ]
        s_ow = [es.enter_context(nc.semaphore(f"s_ow{k}")) for k in range(2)]
        s_hmm = es.enter_context(nc.semaphore("s_hmm"))
        s_hcp = es.enter_context(nc.semaphore("s_hcp"))
        s_s = es.enter_context(nc.semaphore("s_s"))
        s_pmm = es.enter_context(nc.semaphore("s_pmm"))
        s_ocp = es.enter_context(nc.semaphore("s_ocp"))
        block = es.enter_context(nc.Block())

        hw_total = [16 * len(range(k, NHG, 2)) for k in range(2)]

        @block.sync
        def _(sync):
            # one-time loads (single sem: only the total is ever waited on)
            sync.dma_start(w_sb[:, 0, :], w_d[0]).then_inc(s_ld, 16)
            sync.dma_start(w_sb[:, 1, :], w_d[1]).then_inc(s_ld, 16)
            sync.dma_start(on_sb[:, :], on_d[:, :]).then_inc(s_ld, 16)
            sync.dma_start(b_sb[:, :], b_d[:, :]).then_inc(s_ld, 16)
            sync.dma_start(iota_sb[:, :], io_d[:, :]).then_inc(s_ld, 16)
            sync.dma_start(col_sb[:, :], col_d[:, :]).then_inc(s_ld, 16)
            sync.dma_start(rl_sb[:, :], rl_d[:, :]).then_inc(s_ld, 16)
            # phase A: stream xt chunks in, h tile groups out (interleaved)
            ns = 0
            for r, (t0, tn) in enumerate(xt_chunks):
                if r >= 2:
                    pt0, ptn = xt_chunks[r - 2]
                    sync.wait_ge(s_hmm, pt0 + ptn)
                sync.dma_start(
                    xt_sb[:, r % 2, 0:tn, :, :].opt(),
                    xt_d[:, t0 * 256:(t0 + tn) * 256],
                ).then_inc(s_xt[r % 2], 16)
                while ns < NHG and h_groups[ns][0] + h_groups[ns][1] <= t0:
                    g0, gn = h_groups[ns]
                    sync.wait_ge(s_hcp, g0 + gn)
                    sync.dma_start(
                        h_d[g0 * 128:(g0 + gn) * 128, :].rearrange(
                            "(t p) f -> p t f", p=128
                        ),
                        h_sb[:, (ns % 2) * HG:(ns % 2) * HG + gn, :],
                    ).then_inc(s_hw[ns % 2], 16)
                    ns += 1
            while ns < NHG:
                g0, gn = h_groups[ns]
                sync.wait_ge(s_hcp, g0 + gn)
                sync.dma_start(
                    h_d[g0 * 128:(g0 + gn) * 128, :].rearrange(
                        "(t p) f -> p t f", p=128
                    ),
                    h_sb[:, (ns % 2) * HG:(ns % 2) * HG + gn, :],
                ).then_inc(s_hw[ns % 2], 16)
                ns += 1
            # phase B: output stores
            for s in range(NB):
                sync.wait_ge(s_ocp, s + 1)
                sync.dma_start(
                    o_d[s * 128:(s + 1) * 128, :], o_sb[:, s % 2, :]
                ).then_inc(s_ow[s % 2], 16)

        @block.gpsimd
        def _(gpsimd):
            gpsimd.wait_ge(s_ld, 16 * 7)
            for k in range(2):
                gpsimd.wait_ge(s_hw[k], hw_total[k])
            for s in range(NB):
                if s >= 2:
                    gpsimd.wait_ge(s_pmm, mm_cum[s - 1])
                for gi, (g0, gl) in enumerate(subs[s]):
                    off = sub_off[s][gi]
                    gpsimd.dma_gather(
                        val_sb[:, s % 2, g0:g0 + gl, :],
                        h_d[:, :],
                        col_sb[:, off:off + gl * 8],
                        gl * 128,
                        gl * 128,
                        128,
                    ).then_inc(s_gat[(s % 2) * GMAX + gi], 16)

        @block.tensor
        def _(tensor):
            tensor.wait_ge(s_ld, 16 * 7)
            # phase A: h tile i = xt_i^T @ W  (two K chunks)
            for i in range(NT):
                r = chunk_of_tile[i]
                if i == xt_chunks[r][0]:
                    tensor.wait_ge(s_xt[r % 2], 16 * (r // 2 + 1))
                if i >= PSA:
                    tensor.wait_ge(s_hcp, i - (PSA - 1))
                tl = i - xt_chunks[r][0]
                tensor.matmul(
                    ph[i % PSA][:, 0:128],
                    xt_sb[:, r % 2, tl, 0, :],
                    w_sb[:, 0, :],
                    start=True,
                    stop=False,
                )
                tensor.matmul(
                    ph[i % PSA][:, 0:128],
                    xt_sb[:, r % 2, tl, 1, :],
                    w_sb[:, 1, :],
                    start=False,
                    stop=True,
                ).then_inc(s_hmm, 1)
            # phase B: out_slot = ones^T @ bias + sum_c S_c^T @ val_c
            j = 0
            for s in range(NB):
                if s >= 2:
                    tensor.wait_ge(s_ocp, s - 1)
                tensor.matmul(
                    pbk[s % 2][:, 0:128],
                    on_sb[:, :],
                    b_sb[:, :],
                    start=True,
                    stop=False,
                ).then_inc(s_pmm, 1)
                for c in range(K[s]):
                    tensor.wait_ge(s_s, j + 1)
                    tensor.matmul(
                        pbk[s % 2][:, 0:128],
                        s_sb[:, j % SB, :],
                        val_sb[:, s % 2, c, :],
                        start=False,
                        stop=(c == K[s] - 1),
                    ).then_inc(s_pmm, 1)
                    j += 1

        @block.vector
        def _(vector):
            # phase A: PSUM fp32 -> SBUF fp16
            for i in range(NT):
                vector.wait_ge(s_hmm, i + 1)
                g = i // HG
                if g >= 2 and i == g * HG:
                    vector.wait_ge(s_hw[g % 2], 16 * (g // 2))
                vector.tensor_copy(
                    h_sb[:, (g % 2) * HG + (i - g * HG), :],
                    ph[i % PSA][:, 0:128],
                ).then_inc(s_hcp, 1)
            # phase B: one-hot tiles S[e, n] = (iota[n] == rowloc[e])
            jc = 0
            for j in range(KTOT):
                s, gi, g0 = sub_of_chunk[j]
                if jc == 0 or sub_of_chunk[j - 1][:2] != (s, gi):
                    vector.wait_ge(
                        s_gat[(s % 2) * GMAX + gi], 16 * gat_wait[(s, gi)]
                    )
                jc += 1
                if j >= SB:
                    vector.wait_ge(s_pmm, mm_retire[j - SB])
                vector.tensor_scalar(
                    s_sb[:, j % SB, :],
                    iota_sb[:, :],
                    rl_sb[:, j:j + 1],
                    None,
                    mybir.AluOpType.is_equal,
                ).then_inc(s_s, 1)

        @block.scalar
        def _(scalar):
            for s in range(NB):
                scalar.wait_ge(s_pmm, mm_cum[s + 1])
                if s >= 2:
                    scalar.wait_ge(s_ow[s % 2], 16 * (s // 2))
                scalar.activation(
                    o_sb[:, s % 2, :],
                    pbk[s % 2][:, 0:128],
                    mybir.ActivationFunctionType.Relu,
                ).then_inc(s_ocp, 1)

    nc.compile()
    return nc


def _run(x, edge_index, weight, bias, trace=False):
    shared, per_core, plan = _host_prep(x, edge_index, weight, bias)
    nc = _build_program(plan)
    in_maps = [dict(shared, **per_core[c]) for c in range(NCORES)]
    res = run_bass_kernel_spmd(nc, in_maps, list(range(NCORES)), trace=trace)
    out = np.zeros((NPAD, FOUT), np.float32)
    for c in range(NCORES):
        oc = res.results[c]["out"]
        for s in range(NB):
            b = plan["block_of"][c][s]
            if b is not None:
                out[b * 128:(b + 1) * 128] = oc[s * 128:(s + 1) * 128]
    return np.ascontiguousarray(out[:N_NODES]), res


def kernel(x, edge_index, weight, bias):
    out, _ = _run(x, edge_index, weight, bias, trace=False)
    return out


# revision 14
# speedup vs baseline: 1.0141x; 1.0141x over previous
"""GNN message-passing (graph convolution) kernel for 8 Trainium2 NeuronCores.

    out = relu(segment_sum(h[col], row) + bias),  h = x @ W

Strategy (dst-block sharding -- no collectives needed):
  * Host sorts edges by destination node, buckets them into 157 blocks of 128
    dst nodes, sorts blocks by edge count and deals them snake-style into 20
    slots x 8 cores so every core runs the same instruction stream with
    near-balanced per-slot chunk budgets.  Each core produces a disjoint slice
    of the output, so no all-reduce is needed.
  * Phase A (per core, replicated): h = x @ W on the PE in fp16 (PSUM fp32
    accumulate).  x is shipped pre-transposed in a partition-major layout so
    the whole 10 MB streams in as 5 large DMAs (16 KB contiguous per
    partition per DMA -- full DMA-bus rate, HWDGE amortized).  h tiles are
    copied PSUM->SBUF on the DVE and stored to a DRAM buffer h[20096,128]
    fp16 in 16-tile batched DMAs.
  * Phase B: per dst slot, SWDGE dma_gather fetches the slot's (padded) edge
    h rows into SBUF [128e, K, 128f] in sub-gathers of 16 chunks (2048
    descriptors; the SWDGE ring is enlarged via dynamic_dma_scratch_size);
    the DVE builds one-hot tiles S[e,n] = (iota == rowloc) in fp16 (waiting
    on the gather so the PE needs only one wait per matmul); the PE
    accumulates out_block = ones^T @ bias + sum_c S_c^T @ val_c in PSUM fp32
    -- an exact segment-sum with the bias folded in as a K=1 matmul.  ACT
    applies ReLU PSUM->SBUF and results stream out per slot.

Numerics: fp16 operands with fp32 accumulation everywhere; the one-hot matmul
is exact, so the only error is fp16 rounding of x, W and h (~1e-3 relative).
"""

import sys

import numpy as np

sys.path.insert(0, "/opt/trn_rl_repo")

import concourse.bacc as bacc  # noqa: E402
import concourse.bass as bass  # noqa: E402  (engine types)
import concourse.mybir as mybir  # noqa: E402
from concourse.bass_utils import run_bass_kernel_spmd  # noqa: E402

N_NODES = 20000
FIN = 256
FOUT = 128
N_EDGES = 640000

NT = 157                 # node tiles of 128 (nodes padded to 20096)
NPAD = NT * 128          # 20096
NBLK = 157               # dst blocks of 128 nodes
NCORES = 8
NB = 20                  # block slots per core

XT_CH = 32               # xt tiles per load DMA (5 DMAs: 32*4 + 29)
HG = 16                  # h tiles per store DMA (10 DMAs: 16*9 + 13)
SUBG = 8                 # chunks per sub-gather (1024 descriptors)
PSA = 4                  # phase-A psum ring
SB = 4                   # one-hot tile ring

FP16 = mybir.dt.float16
FP32 = mybir.dt.float32
I16 = mybir.dt.int16


def _chunks(total, step):
    out = []
    o = 0
    while o < total:
        out.append((o, min(step, total - o)))
        o += step
    return out


def _host_prep(x, edge_index, weight, bias):
    """Cast/retile operands, bucket + balance edges by destination block.

    Returns (shared, per_core, plan): shared tensors (same on every core),
    per-core col/rl tensors, and the static plan consumed by _build_program.
    """
    x = np.asarray(x, np.float32)
    weight = np.asarray(weight, np.float32)
    bias = np.asarray(bias, np.float32)

    xpad = np.zeros((NPAD, FIN), np.float32)
    xpad[:N_NODES] = x
    # partition-major lhsT: xt[k, i, kc, m] = x[i*128+m, kc*128+k]
    xt = np.ascontiguousarray(
        xpad.reshape(NT, 128, 2, 128).transpose(3, 0, 2, 1).astype(np.float16)
    ).reshape(128, NT * 2 * 128)
    w_t = np.ascontiguousarray(weight.astype(np.float16).reshape(2, 128, 128))
    ones16 = np.ones((1, 128), np.float16)
    bias16 = np.ascontiguousarray(bias.astype(np.float16).reshape(1, 128))
    iota16 = np.ascontiguousarray(
        np.broadcast_to(np.arange(128, dtype=np.float16), (128, 128))
    )

    row = np.asarray(edge_index[0]).astype(np.int64)
    col = np.asarray(edge_index[1]).astype(np.int64)
    order = np.argsort(row, kind="stable")
    rs = row[order].astype(np.int32)
    cs = col[order].astype(np.int32)

    blk = rs >> 7
    counts = np.bincount(blk, minlength=NBLK)
    starts = np.concatenate([[0], np.cumsum(counts)])

    # deal blocks (sorted by descending count) into slots: slot s holds the
    # 8 blocks ranked [8s, 8s+8); per-slot chunk budget = max count in group
    rank = np.argsort(-counts, kind="stable")
    block_of = [[None] * NB for _ in range(NCORES)]
    K = [0] * NB                      # chunks per slot
    for s in range(NB):
        grp = rank[8 * s:8 * s + 8]
        for c, b in enumerate(grp):
            block_of[c][s] = int(b)
        K[s] = max(1, -(-int(counts[grp].max()) // 128))
    KTOT = sum(K)
    subs = [_chunks(K[s], SUBG) for s in range(NB)]       # (chunk_off, len)
    GMAX = max(len(g) for g in subs)

    # per-core index/rowloc buffers
    cidx = 8 * KTOT                    # int16 per partition (idxs: L/16 * 8)
    col16 = np.zeros((NCORES, 128, cidx), np.int16)
    rl16 = np.full((NCORES, 128, KTOT), -1.0, np.float32)
    for core in range(NCORES):
        co = 0
        ko = 0
        for s in range(NB):
            n_pad = K[s] * 128
            lin_col = np.zeros(n_pad, np.int32)
            lin_rl = np.full(n_pad, -1.0, np.float32)
            b = block_of[core][s]
            if b is not None:
                e0, e1 = int(starts[b]), int(starts[b + 1])
                k = e1 - e0
                lin_col[:k] = cs[e0:e1]
                lin_rl[:k] = rs[e0:e1] - b * 128
            for g0, gl in subs[s]:
                L = gl * 128
                piece = lin_col[g0 * 128:g0 * 128 + L]
                col16[core, :, co:co + L // 16] = np.tile(
                    piece.reshape(L // 16, 16).T.astype(np.int16), (8, 1)
                )
                co += L // 16
            rl16[core, :, ko:ko + K[s]] = (
                lin_rl.reshape(K[s], 128).T.astype(np.float32)
            )
            ko += K[s]
        assert co == cidx and ko == KTOT

    shared = {
        "xt": xt,
        "w": w_t,
        "ones": ones16,
        "bias": bias16,
        "iota": iota16,
    }
    per_core = [
        {"col": np.ascontiguousarray(col16[c]),
         "rl": np.ascontiguousarray(rl16[c])}
        for c in range(NCORES)
    ]
    plan = {"K": K, "KTOT": KTOT, "subs": subs, "GMAX": GMAX, "cidx": cidx,
            "block_of": block_of}
    return shared, per_core, plan


def _build_program(plan):
    K, KTOT, subs, GMAX, cidx = (
        plan["K"], plan["KTOT"], plan["subs"], plan["GMAX"], plan["cidx"]
    )
    nc = bacc.Bacc("TRN2", dynamic_dma_scratch_size=16384)

    xt_d = nc.dram_tensor("xt", [128, NT * 2 * 128], FP16, kind="ExternalInput")
    w_d = nc.dram_tensor("w", [2, 128, 128], FP16, kind="ExternalInput")
    on_d = nc.dram_tensor("ones", [1, 128], FP16, kind="ExternalInput")
    b_d = nc.dram_tensor("bias", [1, 128], FP16, kind="ExternalInput")
    io_d = nc.dram_tensor("iota", [128, 128], FP16, kind="ExternalInput")
    col_d = nc.dram_tensor("col", [128, cidx], I16, kind="ExternalInput")
    rl_d = nc.dram_tensor("rl", [128, KTOT], FP32, kind="ExternalInput")
    h_d = nc.dram_tensor("hbuf", [NPAD, 128], FP16)
    o_d = nc.dram_tensor("out", [NB * 128, 128], FP32, kind="ExternalOutput")

    xt_chunks = _chunks(NT, XT_CH)     # (tile_off, n_tiles)
    h_groups = _chunks(NT, HG)
    NXT = len(xt_chunks)
    NHG = len(h_groups)
    chunk_of_tile = []
    for r, (t0, tn) in enumerate(xt_chunks):
        chunk_of_tile += [r] * tn

    KMAX = max(K)
    # retire count of the phase-B matmul that consumes global chunk j
    mm_retire = []
    mm_cum = [0]                       # matmuls through slot s (incl. bias)
    j = 0
    for s in range(NB):
        base = mm_cum[-1] + 1          # bias matmul of slot s
        for c in range(K[s]):
            mm_retire.append(base + c + 1)
        mm_cum.append(base + K[s])
        j += K[s]
    # sub-gather bookkeeping: for global chunk j -> (slot, sub index, first)
    # and the cumulative increment count of each s_gat sem at each use
    sub_of_chunk = []
    gat_count = {}                     # (parity, gi) -> uses so far
    gat_wait = {}                      # (s, gi) -> wait value (x16)
    for s in range(NB):
        for gi, (g0, gl) in enumerate(subs[s]):
            key = (s % 2, gi)
            gat_count[key] = gat_count.get(key, 0) + 1
            gat_wait[(s, gi)] = gat_count[key]
            sub_of_chunk += [(s, gi, g0)] * gl
    # idx (int16-per-partition) offset of each sub-gather, per slot
    sub_off = []
    co = 0
    for s in range(NB):
        offs = []
        for g0, gl in subs[s]:
            offs.append(co)
            co += gl * 8
        sub_off.append(offs)

    from contextlib import ExitStack

    with ExitStack() as es:
        ph = [es.enter_context(nc.psum_tensor(f"ph{k}", [128, 512], FP32))
              for k in range(PSA)]
        pbk = [es.enter_context(nc.psum_tensor(f"pb{k}", [128, 512], FP32))
               for k in range(2)]
        w_sb = es.enter_context(nc.sbuf_tensor("w_sb", [128, 2, 128], FP16))
        on_sb = es.enter_context(nc.sbuf_tensor("on_sb", [1, 128], FP16))
        b_sb = es.enter_context(nc.sbuf_tensor("b_sb", [1, 128], FP16))
        iota_sb = es.enter_context(nc.sbuf_tensor("iota_sb", [128, 128], FP16))
        col_sb = es.enter_context(nc.sbuf_tensor("col_sb", [128, cidx], I16))
        rl_sb = es.enter_context(nc.sbuf_tensor("rl_sb", [128, KTOT], FP32))
        xt_sb = es.enter_context(
            nc.sbuf_tensor("xt_sb", [128, 2, XT_CH, 2, 128], FP16)
        )
        h_sb = es.enter_context(nc.sbuf_tensor("h_sb", [128, 2 * HG, 128], FP16))
        val_sb = es.enter_context(
            nc.sbuf_tensor("val_sb", [128, 2, KMAX, 128], FP16)
        )
        s_sb = es.enter_context(nc.sbuf_tensor("s_sb", [128, SB, 128], FP16))
        o_sb = es.enter_context(nc.sbuf_tensor("o_sb", [128, 2, 128], FP32))

        s_ld = es.enter_context(nc.semaphore("s_ld"))       # one-time loads
        s_xt = [es.enter_context(nc.semaphore(f"s_xt{k}")) for k in range(2)]
        s_hw = [es.enter_context(nc.semaphore(f"s_hw{k}")) for k in range(2)]
        s_gat = [
            es.enter_context(nc.semaphore(f"s_gat{k}"))
            for k in range(2 * GMAX)
        ]
        s_ow = [es.enter_context(nc.semaphore(f"s_ow{k}")) for k in range(2)]
        s_hmm = es.enter_context(nc.semaphore("s_hmm"))
        s_hcp = es.enter_context(nc.semaphore("s_hcp"))
        s_s = es.enter_context(nc.semaphore("s_s"))
        s_pmm = es.enter_context(nc.semaphore("s_pmm"))
        s_ocp = es.enter_context(nc.semaphore("s_ocp"))
        block = es.enter_context(nc.Block())

        hw_total = [16 * len(range(k, NHG, 2)) for k in range(2)]

        @block.sync
        def _(sync):
            # one-time loads (single sem: only the total is ever waited on)
            sync.dma_start(w_sb[:, 0, :], w_d[0]).then_inc(s_ld, 16)
            sync.dma_start(w_sb[:, 1, :], w_d[1]).then_inc(s_ld, 16)
            sync.dma_start(on_sb[:, :], on_d[:, :]).then_inc(s_ld, 16)
            sync.dma_start(b_sb[:, :], b_d[:, :]).then_inc(s_ld, 16)
            sync.dma_start(iota_sb[:, :], io_d[:, :]).then_inc(s_ld, 16)
            sync.dma_start(col_sb[:, :], col_d[:, :]).then_inc(s_ld, 16)
            sync.dma_start(rl_sb[:, :], rl_d[:, :]).then_inc(s_ld, 16)
            # phase A: stream xt chunks in, h tile groups out (interleaved)
            ns = 0
            for r, (t0, tn) in enumerate(xt_chunks):
                if r >= 2:
                    pt0, ptn = xt_chunks[r - 2]
                    sync.wait_ge(s_hmm, pt0 + ptn)
                sync.dma_start(
                    xt_sb[:, r % 2, 0:tn, :, :].opt(),
                    xt_d[:, t0 * 256:(t0 + tn) * 256],
                ).then_inc(s_xt[r % 2], 16)
                while ns < NHG and h_groups[ns][0] + h_groups[ns][1] <= t0:
                    g0, gn = h_groups[ns]
                    sync.wait_ge(s_hcp, g0 + gn)
                    sync.dma_start(
                        h_d[g0 * 128:(g0 + gn) * 128, :].rearrange(
                            "(t p) f -> p t f", p=128
                        ),
                        h_sb[:, (ns % 2) * HG:(ns % 2) * HG + gn, :],
                    ).then_inc(s_hw[ns % 2], 16)
                    ns += 1
            while ns < NHG:
                g0, gn = h_groups[ns]
                sync.wait_ge(s_hcp, g0 + gn)
                sync.dma_start(
                    h_d[g0 * 128:(g0 + gn) * 128, :].rearrange(
                        "(t p) f -> p t f", p=128
                    ),
                    h_sb[:, (ns % 2) * HG:(ns % 2) * HG + gn, :],
                ).then_inc(s_hw[ns % 2], 16)
                ns += 1
            # phase B: output stores
            for s in range(NB):
                sync.wait_ge(s_ocp, s + 1)
                sync.dma_start(
                    o_d[s * 128:(s + 1) * 128, :], o_sb[:, s % 2, :]
                ).then_inc(s_ow[s % 2], 16)

        @block.gpsimd
        def _(gpsimd):
            gpsimd.wait_ge(s_ld, 16 * 7)
            for k in range(2):
                gpsimd.wait_ge(s_hw[k], hw_total[k])
            for s in range(NB):
                if s >= 2:
                    gpsimd.wait_ge(s_pmm, mm_cum[s - 1])
                for gi, (g0, gl) in enumerate(subs[s]):
                    off = sub_off[s][gi]
                    gpsimd.dma_gather(
                        val_sb[:, s % 2, g0:g0 + gl, :],
                        h_d[:, :],
                        col_sb[:, off:off + gl * 8],
                        gl * 128,
                        gl * 128,
                        128,
                    ).then_inc(s_gat[(s % 2) * GMAX + gi], 16)

        @block.tensor
        def _(tensor):
            tensor.wait_ge(s_ld, 16 * 7)
            # phase A: h tile i = xt_i^T @ W  (two K chunks)
            for i in range(NT):
                r = chunk_of_tile[i]
                if i == xt_chunks[r][0]:
                    tensor.wait_ge(s_xt[r % 2], 16 * (r // 2 + 1))
                if i >= PSA:
                    tensor.wait_ge(s_hcp, i - (PSA - 1))
                tl = i - xt_chunks[r][0]
                tensor.matmul(
                    ph[i % PSA][:, 0:128],
                    xt_sb[:, r % 2, tl, 0, :],
                    w_sb[:, 0, :],
                    start=True,
                    stop=False,
                )
                tensor.matmul(
                    ph[i % PSA][:, 0:128],
                    xt_sb[:, r % 2, tl, 1, :],
                    w_sb[:, 1, :],
                    start=False,
                    stop=True,
                ).then_inc(s_hmm, 1)
            # phase B: out_slot = ones^T @ bias + sum_c S_c^T @ val_c
            j = 0
            for s in range(NB):
                if s >= 2:
                    tensor.wait_ge(s_ocp, s - 1)
                tensor.matmul(
                    pbk[s % 2][:, 0:128],
                    on_sb[:, :],
                    b_sb[:, :],
                    start=True,
                    stop=False,
                ).then_inc(s_pmm, 1)
                for c in range(K[s]):
                    tensor.wait_ge(s_s, j + 1)
                    tensor.matmul(
                        pbk[s % 2][:, 0:128],
                        s_sb[:, j % SB, :],
                        val_sb[:, s % 2, c, :],
                        start=False,
                        stop=(c == K[s] - 1),
                    ).then_inc(s_pmm, 1)
                    j += 1

        @block.vector
        def _(vector):
            # phase A: PSUM fp32 -> SBUF fp16
            for i in range(NT):
                vector.wait_ge(s_hmm, i + 1)
                g = i // HG
                if g >= 2 and i == g * HG:
                    vector.wait_ge(s_hw[g % 2], 16 * (g // 2))
                vector.tensor_copy(
                    h_sb[:, (g % 2) * HG + (i - g * HG), :],
                    ph[i % PSA][:, 0:128],
                ).then_inc(s_hcp, 1)
            # phase B: one-hot tiles S[e, n] = (iota[n] == rowloc[e])
            jc = 0
            for j in range(KTOT):
                s, gi, g0 = sub_of_chunk[j]
                if jc == 0 or sub_of_chunk[j - 1][:2] != (s, gi):
                    vector.wait_ge(
                        s_gat[(s % 2) * GMAX + gi], 16 * gat_wait[(s, gi)]
                    )
                jc += 1
                if j >= SB:
                    vector.wait_ge(s_pmm, mm_retire[j - SB])
                vector.tensor_scalar(
                    s_sb[:, j % SB, :],
                    iota_sb[:, :],
                    rl_sb[:, j:j + 1],
                    None,
                    mybir.AluOpType.is_equal,
                ).then_inc(s_s, 1)

        @block.scalar
        def _(scalar):
            for s in range(NB):
                scalar.wait_ge(s_pmm, mm_cum[s + 1])
                if s >= 2:
                    scalar.wait_ge(s_ow[s % 2], 16 * (s // 2))
                scalar.activation(
                    o_sb[:, s % 2, :],
                    pbk[s % 2][:, 0:128],
                    mybir.ActivationFunctionType.Relu,
                ).then_inc(s_ocp, 1)

    nc.compile()
    return nc


def _run(x, edge_index, weight, bias, trace=False):
    shared, per_core, plan = _host_prep(x, edge_index, weight, bias)
    nc = _build_program(plan)
    in_maps = [dict(shared, **per_core[c]) for c in range(NCORES)]
    res = run_bass_kernel_spmd(nc, in_maps, list(range(NCORES)), trace=trace)
    out = np.zeros((NPAD, FOUT), np.float32)
    for c in range(NCORES):
        oc = res.results[c]["out"]
        for s in range(NB):
            b = plan["block_of"][c][s]
            if b is not None:
                out[b * 128:(b + 1) * 128] = oc[s * 128:(s + 1) * 128]
    return np.ascontiguousarray(out[:N_NODES]), res


def kernel(x, edge_index, weight, bias):
    out, _ = _run(x, edge_index, weight, bias, trace=False)
    return out


# revision 26
# speedup vs baseline: 1.0551x; 1.0404x over previous
"""GNN message-passing (graph convolution) kernel for 8 Trainium2 NeuronCores.

    out = relu(segment_sum(h[col], row) + bias),  h = x @ W

Strategy (dst-block sharding -- no collectives needed):
  * Host sorts edges by destination node, buckets them into 157 blocks of 128
    dst nodes, sorts blocks by edge count and deals them snake-style into 20
    slots x 8 cores so every core runs the same instruction stream with
    near-balanced per-slot chunk budgets.  Each core produces a disjoint slice
    of the output, so no all-reduce is needed.
  * Phase A (per core, replicated): h = x @ W on the PE in fp16 (PSUM fp32
    accumulate).  x is shipped pre-transposed in a partition-major layout so
    the whole 10 MB streams in as 5 large DMAs (16 KB contiguous per
    partition per DMA -- full DMA-bus rate, HWDGE amortized).  h tiles are
    copied PSUM->SBUF on the DVE and stored to a DRAM buffer h[20096,128]
    fp16 in 16-tile batched DMAs.
  * Phase B: per dst slot, SWDGE dma_gather fetches the slot's (padded) edge
    h rows into SBUF [128e, K, 128f] in sub-gathers of 16 chunks (2048
    descriptors; the SWDGE ring is enlarged via dynamic_dma_scratch_size);
    the DVE builds one-hot tiles S[e,n] = (iota == rowloc) in fp16 (waiting
    on the gather so the PE needs only one wait per matmul); the PE
    accumulates out_block = ones^T @ bias + sum_c S_c^T @ val_c in PSUM fp32
    -- an exact segment-sum with the bias folded in as a K=1 matmul.  ACT
    applies ReLU PSUM->SBUF and results stream out per slot.

Numerics: fp16 operands with fp32 accumulation everywhere; the one-hot matmul
is exact, so the only error is fp16 rounding of x, W and h (~1e-3 relative).
"""

import sys

import numpy as np

sys.path.insert(0, "/opt/trn_rl_repo")

import concourse.bacc as bacc  # noqa: E402
import concourse.bass as bass  # noqa: E402  (engine types)
import concourse.mybir as mybir  # noqa: E402
from concourse.bass_utils import run_bass_kernel_spmd  # noqa: E402

N_NODES = 20000
FIN = 256
FOUT = 128
N_EDGES = 640000

NT = 157                 # node tiles of 128 (nodes padded to 20096)
NPAD = NT * 128          # 20096
NBLK = 157               # dst blocks of 128 nodes
NCORES = 8
NB = 20                  # block slots per core

XT_CH = 20               # xt tiles per load DMA (8 DMAs: 20*7 + 17)
HG = 16                  # h tiles per store DMA (10 DMAs: 16*9 + 13)
SUBG = 8                 # chunks per sub-gather (1024 descriptors)
PSA = 4                  # phase-A psum ring
SB = 6                   # one-hot tile ring
HRING = 3                # h store group ring depth
VR = 64                  # val chunk ring (multiple of SUBG)
NGS = 8                  # gather completion sem rotation

FP16 = mybir.dt.float16
FP32 = mybir.dt.float32
I16 = mybir.dt.int16


def _chunks(total, step):
    out = []
    o = 0
    while o < total:
        out.append((o, min(step, total - o)))
        o += step
    return out


def _host_prep(x, edge_index, weight, bias):
    """Cast/retile operands, bucket + balance edges by destination block.

    Returns (shared, per_core, plan): shared tensors (same on every core),
    per-core col/rl tensors, and the static plan consumed by _build_program.
    """
    x = np.asarray(x, np.float32)
    weight = np.asarray(weight, np.float32)
    bias = np.asarray(bias, np.float32)

    xpad = np.zeros((NPAD, FIN), np.float32)
    xpad[:N_NODES] = x
    # partition-major lhsT: xt[k, i, kc, m] = x[i*128+m, kc*128+k]
    xt = np.ascontiguousarray(
        xpad.reshape(NT, 128, 2, 128).transpose(3, 0, 2, 1).astype(np.float16)
    ).reshape(128, NT * 2 * 128)
    w_t = np.ascontiguousarray(weight.astype(np.float16).reshape(2, 128, 128))
    ones16 = np.ones((1, 128), np.float16)
    bias16 = np.ascontiguousarray(bias.astype(np.float16).reshape(1, 128))
    iota16 = np.ascontiguousarray(
        np.broadcast_to(np.arange(128, dtype=np.float16), (128, 128))
    )

    row = np.asarray(edge_index[0]).astype(np.int64)
    col = np.asarray(edge_index[1]).astype(np.int64)
    order = np.argsort(row, kind="stable")
    rs = row[order].astype(np.int32)
    cs = col[order].astype(np.int32)

    blk = rs >> 7
    counts = np.bincount(blk, minlength=NBLK)
    starts = np.concatenate([[0], np.cumsum(counts)])

    # deal blocks (sorted by descending count) into slots: slot s holds the
    # 8 blocks ranked [8s, 8s+8); per-slot chunk budget = max count in group
    rank = np.argsort(-counts, kind="stable")
    block_of = [[None] * NB for _ in range(NCORES)]
    K = [0] * NB                      # chunks per slot
    n_ex = [0] * NB                   # exact (16-rounded) idxs per slot
    for s in range(NB):
        grp = rank[8 * s:8 * s + 8]
        for c, b in enumerate(grp):
            block_of[c][s] = int(b)
        n_ex[s] = max(16, -(-int(counts[grp].max()) // 16) * 16)
        K[s] = -(-n_ex[s] // 128)
    KTOT = sum(K)
    # flat chunk stream: slot boundaries only matter to the matmul schedule;
    # gathers run in cross-slot windows of SUBG full chunks
    wins = _chunks(KTOT, SUBG)                            # (chunk_off, len)

    # per-core index/rowloc buffers (idx stream wrapped per window)
    cidx = 8 * KTOT                    # int16 per partition
    col16 = np.zeros((NCORES, 128, cidx), np.int16)
    rl16 = np.full((NCORES, 128, KTOT), -1.0, np.float32)
    for core in range(NCORES):
        lin_col = np.zeros(KTOT * 128, np.int32)
        ko = 0
        for s in range(NB):
            n_pad = K[s] * 128
            lin_rl = np.full(n_pad, -1.0, np.float32)
            b = block_of[core][s]
            if b is not None:
                e0, e1 = int(starts[b]), int(starts[b + 1])
                k = e1 - e0
                lin_col[ko * 128:ko * 128 + k] = cs[e0:e1]
                lin_rl[:k] = rs[e0:e1] - b * 128
            rl16[core, :, ko:ko + K[s]] = (
                lin_rl.reshape(K[s], 128).T.astype(np.float32)
            )
            ko += K[s]
        assert ko == KTOT
        for j0, jn in wins:
            L = jn * 128
            piece = lin_col[j0 * 128:j0 * 128 + L]
            col16[core, :, j0 * 8:j0 * 8 + L // 16] = np.tile(
                piece.reshape(L // 16, 16).T.astype(np.int16), (8, 1)
            )

    shared = {
        "xt": xt,
        "w": w_t,
        "ones": ones16,
        "bias": bias16,
        "iota": iota16,
    }
    per_core = [
        {"col": np.ascontiguousarray(col16[c]),
         "rl": np.ascontiguousarray(rl16[c])}
        for c in range(NCORES)
    ]
    plan = {"K": K, "KTOT": KTOT, "wins": wins, "cidx": cidx,
            "block_of": block_of}
    return shared, per_core, plan


def _build_program(plan):
    K, KTOT, wins, cidx = (
        plan["K"], plan["KTOT"], plan["wins"], plan["cidx"]
    )
    nc = bacc.Bacc("TRN2", dynamic_dma_scratch_size=16384)

    xt_d = nc.dram_tensor("xt", [128, NT * 2 * 128], FP16, kind="ExternalInput")
    w_d = nc.dram_tensor("w", [2, 128, 128], FP16, kind="ExternalInput")
    on_d = nc.dram_tensor("ones", [1, 128], FP16, kind="ExternalInput")
    b_d = nc.dram_tensor("bias", [1, 128], FP16, kind="ExternalInput")
    io_d = nc.dram_tensor("iota", [128, 128], FP16, kind="ExternalInput")
    col_d = nc.dram_tensor("col", [128, cidx], I16, kind="ExternalInput")
    rl_d = nc.dram_tensor("rl", [128, KTOT], FP32, kind="ExternalInput")
    h_d = nc.dram_tensor("hbuf", [NPAD, 128], FP16)
    o_d = nc.dram_tensor("out", [NB * 128, 128], FP32, kind="ExternalOutput")

    xt_chunks = _chunks(NT, XT_CH)     # (tile_off, n_tiles)
    h_groups = _chunks(NT, HG)
    NXT = len(xt_chunks)
    NHG = len(h_groups)
    chunk_of_tile = []
    for r, (t0, tn) in enumerate(xt_chunks):
        chunk_of_tile += [r] * tn

    NW = len(wins)
    # retire count of the phase-B matmul that consumes global chunk j
    mm_retire = []
    mm_cum = [0]                       # matmuls through slot s (incl. bias)
    for s in range(NB):
        base = mm_cum[-1] + 1          # bias matmul of slot s
        for c in range(K[s]):
            mm_retire.append(base + c + 1)
        mm_cum.append(base + K[s])
    win_of_chunk = []
    for w, (j0, jn) in enumerate(wins):
        win_of_chunk += [w] * jn

    from contextlib import ExitStack

    with ExitStack() as es:
        ph = [es.enter_context(nc.psum_tensor(f"ph{k}", [128, 512], FP32))
              for k in range(PSA)]
        pbk = [es.enter_context(nc.psum_tensor(f"pb{k}", [128, 512], FP32))
               for k in range(2)]
        w_sb = es.enter_context(nc.sbuf_tensor("w_sb", [128, 2, 128], FP16))
        on_sb = es.enter_context(nc.sbuf_tensor("on_sb", [1, 128], FP16))
        b_sb = es.enter_context(nc.sbuf_tensor("b_sb", [1, 128], FP16))
        iota_sb = es.enter_context(nc.sbuf_tensor("iota_sb", [128, 128], FP16))
        col_sb = es.enter_context(nc.sbuf_tensor("col_sb", [128, cidx], I16))
        rl_sb = es.enter_context(nc.sbuf_tensor("rl_sb", [128, KTOT], FP32))
        xt_sb = es.enter_context(
            nc.sbuf_tensor("xt_sb", [128, 2, XT_CH, 2, 128], FP16)
        )
        h_sb = es.enter_context(
            nc.sbuf_tensor("h_sb", [128, HRING * HG, 128], FP16)
        )
        val_sb = es.enter_context(
            nc.sbuf_tensor("val_sb", [128, VR, 128], FP16)
        )
        s_sb = es.enter_context(nc.sbuf_tensor("s_sb", [128, SB, 128], FP16))
        o_sb = es.enter_context(nc.sbuf_tensor("o_sb", [128, 2, 128], FP32))

        s_ld = es.enter_context(nc.semaphore("s_ld"))       # one-time loads
        s_ldw = es.enter_context(nc.semaphore("s_ldw"))     # weight loads
        s_xt = [es.enter_context(nc.semaphore(f"s_xt{k}")) for k in range(2)]
        s_hw = [
            es.enter_context(nc.semaphore(f"s_hw{k}")) for k in range(HRING)
        ]
        s_gat = [
            es.enter_context(nc.semaphore(f"s_gat{k}")) for k in range(NGS)
        ]
        s_ow = [es.enter_context(nc.semaphore(f"s_ow{k}")) for k in range(2)]
        s_hmm = es.enter_context(nc.semaphore("s_hmm"))
        s_hcp = es.enter_context(nc.semaphore("s_hcp"))
        s_s = es.enter_context(nc.semaphore("s_s"))
        s_pmm = es.enter_context(nc.semaphore("s_pmm"))
        s_ocp = es.enter_context(nc.semaphore("s_ocp"))
        block = es.enter_context(nc.Block())

        hw_total = [16 * len(range(k, NHG, HRING)) for k in range(HRING)]

        @block.sync
        def _(sync):
            # phase A first: the PE only needs xt chunk 0 + W to start, so
            # those go ahead of the bulky phase-B tables (col is 1.3 MB)
            ns = 0
            for r, (t0, tn) in enumerate(xt_chunks):
                if r == 1:
                    sync.dma_start(w_sb[:, 0, :], w_d[0]).then_inc(s_ldw, 16)
                    sync.dma_start(w_sb[:, 1, :], w_d[1]).then_inc(s_ldw, 16)
                elif r == 2:
                    sync.dma_start(iota_sb[:, :], io_d[:, :]).then_inc(
                        s_ld, 16
                    )
                    sync.dma_start(rl_sb[:, :], rl_d[:, :]).then_inc(s_ld, 16)
                    sync.dma_start(col_sb[:, :], col_d[:, :]).then_inc(
                        s_ld, 16
                    )
                    sync.dma_start(on_sb[:, :], on_d[:, :]).then_inc(s_ld, 16)
                    sync.dma_start(b_sb[:, :], b_d[:, :]).then_inc(s_ld, 16)
                if r >= 2:
                    pt0, ptn = xt_chunks[r - 2]
                    sync.wait_ge(s_hmm, pt0 + ptn)
                sync.dma_start(
                    xt_sb[:, r % 2, 0:tn, :, :].opt(),
                    xt_d[:, t0 * 256:(t0 + tn) * 256],
                ).then_inc(s_xt[r % 2], 16)
                while ns < NHG and h_groups[ns][0] + h_groups[ns][1] <= t0:
                    g0, gn = h_groups[ns]
                    sync.wait_ge(s_hcp, g0 + gn)
                    sync.dma_start(
                        h_d[g0 * 128:(g0 + gn) * 128, :].rearrange(
                            "(t p) f -> p t f", p=128
                        ),
                        h_sb[:, (ns % HRING) * HG:(ns % HRING) * HG + gn, :],
                    ).then_inc(s_hw[ns % HRING], 16)
                    ns += 1
            while ns < NHG:
                g0, gn = h_groups[ns]
                sync.wait_ge(s_hcp, g0 + gn)
                sync.dma_start(
                    h_d[g0 * 128:(g0 + gn) * 128, :].rearrange(
                        "(t p) f -> p t f", p=128
                    ),
                    h_sb[:, (ns % HRING) * HG:(ns % HRING) * HG + gn, :],
                ).then_inc(s_hw[ns % HRING], 16)
                ns += 1
            # phase B: output stores
            for s in range(NB):
                sync.wait_ge(s_ocp, s + 1)
                sync.dma_start(
                    o_d[s * 128:(s + 1) * 128, :], o_sb[:, s % 2, :]
                ).then_inc(s_ow[s % 2], 16)

        @block.gpsimd
        def _(gpsimd):
            gpsimd.wait_ge(s_ld, 16 * 5)
            for k in range(HRING):
                gpsimd.wait_ge(s_hw[k], hw_total[k])
            for w, (j0, jn) in enumerate(wins):
                j1 = j0 + jn
                if j1 > VR:
                    gpsimd.wait_ge(s_pmm, mm_retire[j1 - VR - 1])
                gpsimd.dma_gather(
                    val_sb[:, j0 % VR:j0 % VR + jn, :],
                    h_d[:, :],
                    col_sb[:, j0 * 8:j1 * 8],
                    jn * 128,
                    jn * 128,
                    128,
                ).then_inc(s_gat[w % NGS], 16)

        @block.tensor
        def _(tensor):
            tensor.wait_ge(s_ldw, 32)
            # phase A: h tile i = xt_i^T @ W  (two K chunks)
            for i in range(NT):
                r = chunk_of_tile[i]
                if i == xt_chunks[r][0]:
                    tensor.wait_ge(s_xt[r % 2], 16 * (r // 2 + 1))
                if i >= PSA:
                    tensor.wait_ge(s_hcp, i - (PSA - 1))
                tl = i - xt_chunks[r][0]
                tensor.matmul(
                    ph[i % PSA][:, 0:128],
                    xt_sb[:, r % 2, tl, 0, :],
                    w_sb[:, 0, :],
                    start=True,
                    stop=False,
                )
                tensor.matmul(
                    ph[i % PSA][:, 0:128],
                    xt_sb[:, r % 2, tl, 1, :],
                    w_sb[:, 1, :],
                    start=False,
                    stop=True,
                ).then_inc(s_hmm, 1)
            # phase B: out_slot = ones^T @ bias + sum_c S_c^T @ val_c
            tensor.wait_ge(s_ld, 16 * 5)
            j = 0
            for s in range(NB):
                if s >= 2:
                    tensor.wait_ge(s_ocp, s - 1)
                tensor.matmul(
                    pbk[s % 2][:, 0:128],
                    on_sb[:, :],
                    b_sb[:, :],
                    start=True,
                    stop=False,
                ).then_inc(s_pmm, 1)
                for c in range(K[s]):
                    tensor.wait_ge(s_s, j + 1)
                    tensor.matmul(
                        pbk[s % 2][:, 0:128],
                        s_sb[:, j % SB, :],
                        val_sb[:, j % VR, :],
                        start=False,
                        stop=(c == K[s] - 1),
                    ).then_inc(s_pmm, 1)
                    j += 1

        @block.vector
        def _(vector):
            # phase A: PSUM fp32 -> SBUF fp16
            for i in range(NT):
                vector.wait_ge(s_hmm, i + 1)
                g = i // HG
                if g >= HRING and i == g * HG:
                    vector.wait_ge(s_hw[g % HRING], 16 * (g // HRING))
                vector.tensor_copy(
                    h_sb[:, (g % HRING) * HG + (i - g * HG), :],
                    ph[i % PSA][:, 0:128],
                ).then_inc(s_hcp, 1)
            # phase B: one-hot tiles S[e, n] = (iota[n] == rowloc[e])
            for j in range(KTOT):
                w = win_of_chunk[j]
                if j == 0 or win_of_chunk[j - 1] != w:
                    vector.wait_ge(s_gat[w % NGS], 16 * (w // NGS + 1))
                if j >= SB:
                    vector.wait_ge(s_pmm, mm_retire[j - SB])
                vector.tensor_scalar(
                    s_sb[:, j % SB, :],
                    iota_sb[:, :],
                    rl_sb[:, j:j + 1],
                    None,
                    mybir.AluOpType.is_equal,
                ).then_inc(s_s, 1)

        @block.scalar
        def _(scalar):
            for s in range(NB):
                scalar.wait_ge(s_pmm, mm_cum[s + 1])
                if s >= 2:
                    scalar.wait_ge(s_ow[s % 2], 16 * (s // 2))
                scalar.activation(
                    o_sb[:, s % 2, :],
                    pbk[s % 2][:, 0:128],
                    mybir.ActivationFunctionType.Relu,
                ).then_inc(s_ocp, 1)

    nc.compile()
    return nc


def _run(x, edge_index, weight, bias, trace=False):
    shared, per_core, plan = _host_prep(x, edge_index, weight, bias)
    nc = _build_program(plan)
    in_maps = [dict(shared, **per_core[c]) for c in range(NCORES)]
    res = run_bass_kernel_spmd(nc, in_maps, list(range(NCORES)), trace=trace)
    out = np.zeros((NPAD, FOUT), np.float32)
    for c in range(NCORES):
        oc = res.results[c]["out"]
        for s in range(NB):
            b = plan["block_of"][c][s]
            if b is not None:
                out[b * 128:(b + 1) * 128] = oc[s * 128:(s + 1) * 128]
    return np.ascontiguousarray(out[:N_NODES]), res


def kernel(x, edge_index, weight, bias):
    out, _ = _run(x, edge_index, weight, bias, trace=False)
    return out


# revision 31
# speedup vs baseline: 1.1046x; 1.0469x over previous
"""GNN message-passing (graph convolution) kernel for 8 Trainium2 NeuronCores.

    out = relu(segment_sum(h[col], row) + bias),  h = x @ W

Strategy (dst-block sharding -- no collectives needed):
  * Host sorts edges by destination node, buckets them into 157 blocks of 128
    dst nodes, sorts blocks by edge count and deals them snake-style into 20
    slots x 8 cores so every core runs the same instruction stream with
    near-balanced per-slot chunk budgets.  Each core produces a disjoint slice
    of the output, so no all-reduce is needed.
  * Phase A (per core, replicated): h = x @ W on the PE in fp16 (PSUM fp32
    accumulate).  x is shipped pre-transposed in a partition-major layout so
    the whole 10 MB streams in as 5 large DMAs (16 KB contiguous per
    partition per DMA -- full DMA-bus rate, HWDGE amortized).  h tiles are
    copied PSUM->SBUF on the DVE and stored to a DRAM buffer h[20096,128]
    fp16 in 16-tile batched DMAs.
  * Phase B: per dst slot, SWDGE dma_gather fetches the slot's (padded) edge
    h rows into SBUF [128e, K, 128f] in sub-gathers of 16 chunks (2048
    descriptors; the SWDGE ring is enlarged via dynamic_dma_scratch_size);
    the DVE builds one-hot tiles S[e,n] = (iota == rowloc) in fp16 (waiting
    on the gather so the PE needs only one wait per matmul); the PE
    accumulates out_block = ones^T @ bias + sum_c S_c^T @ val_c in PSUM fp32
    -- an exact segment-sum with the bias folded in as a K=1 matmul.  ACT
    applies ReLU PSUM->SBUF and results stream out per slot.

Numerics: fp16 operands with fp32 accumulation everywhere; the one-hot matmul
is exact, so the only error is fp16 rounding of x, W and h (~1e-3 relative).
"""

import sys

import numpy as np

sys.path.insert(0, "/opt/trn_rl_repo")

import concourse.bacc as bacc  # noqa: E402
import concourse.bass as bass  # noqa: E402  (engine types)
import concourse.mybir as mybir  # noqa: E402
from concourse.bass_utils import run_bass_kernel_spmd  # noqa: E402

N_NODES = 20000
FIN = 256
FOUT = 128
N_EDGES = 640000

NT = 157                 # node tiles of 128 (nodes padded to 20096)
NPAD = NT * 128          # 20096
NBLK = 157               # dst blocks of 128 nodes
NCORES = 8
NB = 20                  # block slots per core

XT_CH = 20               # xt tiles per load DMA (8 DMAs: 20*7 + 17)
HG = 16                  # h tiles per store DMA (10 DMAs: 16*9 + 13)
SUBG = 8                 # chunks per sub-gather (1024 descriptors)
PSA = 4                  # phase-A psum ring
SB = 6                   # one-hot tile ring
HRING = 3                # h store group ring depth
VR = 64                  # val chunk ring (multiple of SUBG)
NGS = 8                  # gather completion sem rotation

FP16 = mybir.dt.float16
FP32 = mybir.dt.float32
I16 = mybir.dt.int16


def _chunks(total, step):
    out = []
    o = 0
    while o < total:
        out.append((o, min(step, total - o)))
        o += step
    return out


def _host_prep(x, edge_index, weight, bias):
    """Cast/retile operands, bucket + balance edges by destination block.

    Returns (shared, per_core, plan): shared tensors (same on every core),
    per-core col/rl tensors, and the static plan consumed by _build_program.
    """
    x = np.asarray(x, np.float32)
    weight = np.asarray(weight, np.float32)
    bias = np.asarray(bias, np.float32)

    xpad = np.zeros((NPAD, FIN), np.float32)
    xpad[:N_NODES] = x
    # partition-major lhsT: xt[k, i, kc, m] = x[i*128+m, kc*128+k]
    xt = np.ascontiguousarray(
        xpad.reshape(NT, 128, 2, 128).transpose(3, 0, 2, 1).astype(np.float16)
    ).reshape(128, NT * 2 * 128)
    w_t = np.ascontiguousarray(weight.astype(np.float16).reshape(2, 128, 128))
    ones16 = np.ones((1, 128), np.float16)
    bias16 = np.ascontiguousarray(bias.astype(np.float16).reshape(1, 128))
    iota16 = np.ascontiguousarray(
        np.broadcast_to(np.arange(128, dtype=np.float16), (128, 128))
    )

    row = np.asarray(edge_index[0]).astype(np.int64)
    col = np.asarray(edge_index[1]).astype(np.int64)
    order = np.argsort(row, kind="stable")
    rs = row[order].astype(np.int32)
    cs = col[order].astype(np.int32)

    blk = rs >> 7
    counts = np.bincount(blk, minlength=NBLK)
    starts = np.concatenate([[0], np.cumsum(counts)])

    # deal blocks (sorted by descending count) into slots: slot s holds the
    # 8 blocks ranked [8s, 8s+8); per-slot chunk budget = max count in group
    rank = np.argsort(-counts, kind="stable")
    block_of = [[None] * NB for _ in range(NCORES)]
    K = [0] * NB                      # chunks per slot
    n_ex = [0] * NB                   # exact (16-rounded) idxs per slot
    for s in range(NB):
        grp = rank[8 * s:8 * s + 8]
        for c, b in enumerate(grp):
            block_of[c][s] = int(b)
        n_ex[s] = max(16, -(-int(counts[grp].max()) // 16) * 16)
        K[s] = -(-n_ex[s] // 128)
    KTOT = sum(K)
    # flat chunk stream: slot boundaries only matter to the matmul schedule;
    # gathers run in cross-slot windows of SUBG full chunks
    wins = _chunks(KTOT, SUBG)                            # (chunk_off, len)

    # per-core index/rowloc buffers (idx stream wrapped per window)
    cidx = 8 * KTOT                    # int16 per partition
    col16 = np.zeros((NCORES, 128, cidx), np.int16)
    rl16 = np.full((NCORES, 128, KTOT), -1.0, np.float32)
    for core in range(NCORES):
        lin_col = np.zeros(KTOT * 128, np.int32)
        ko = 0
        for s in range(NB):
            n_pad = K[s] * 128
            lin_rl = np.full(n_pad, -1.0, np.float32)
            b = block_of[core][s]
            if b is not None:
                e0, e1 = int(starts[b]), int(starts[b + 1])
                k = e1 - e0
                cse = cs[e0:e1]
                # pair-interleaved h row: u = t2*256 + 2p + half
                lin_col[ko * 128:ko * 128 + k] = (
                    (cse >> 8 << 8) + ((cse & 127) << 1) + ((cse >> 7) & 1)
                )
                lin_rl[:k] = rs[e0:e1] - b * 128
            rl16[core, :, ko:ko + K[s]] = (
                lin_rl.reshape(K[s], 128).T.astype(np.float32)
            )
            ko += K[s]
        assert ko == KTOT
        for j0, jn in wins:
            L = jn * 128
            piece = lin_col[j0 * 128:j0 * 128 + L]
            col16[core, :, j0 * 8:j0 * 8 + L // 16] = np.tile(
                piece.reshape(L // 16, 16).T.astype(np.int16), (8, 1)
            )

    shared = {
        "xt": xt,
        "w": w_t,
        "ones": ones16,
        "bias": bias16,
        "iota": iota16,
    }
    per_core = [
        {"col": np.ascontiguousarray(col16[c]),
         "rl": np.ascontiguousarray(rl16[c])}
        for c in range(NCORES)
    ]
    plan = {"K": K, "KTOT": KTOT, "wins": wins, "cidx": cidx,
            "block_of": block_of}
    return shared, per_core, plan


def _build_program(plan):
    K, KTOT, wins, cidx = (
        plan["K"], plan["KTOT"], plan["wins"], plan["cidx"]
    )
    nc = bacc.Bacc("TRN2", dynamic_dma_scratch_size=16384)

    xt_d = nc.dram_tensor("xt", [128, NT * 2 * 128], FP16, kind="ExternalInput")
    w_d = nc.dram_tensor("w", [2, 128, 128], FP16, kind="ExternalInput")
    on_d = nc.dram_tensor("ones", [1, 128], FP16, kind="ExternalInput")
    b_d = nc.dram_tensor("bias", [1, 128], FP16, kind="ExternalInput")
    io_d = nc.dram_tensor("iota", [128, 128], FP16, kind="ExternalInput")
    col_d = nc.dram_tensor("col", [128, cidx], I16, kind="ExternalInput")
    rl_d = nc.dram_tensor("rl", [128, KTOT], FP32, kind="ExternalInput")
    h_d = nc.dram_tensor("hbuf", [(NT + 1) // 2 * 256, 128], FP16)
    o_d = nc.dram_tensor("out", [NB * 128, 128], FP32, kind="ExternalOutput")

    xt_chunks = _chunks(NT, XT_CH)     # (tile_off, n_tiles)
    h_groups = _chunks(NT, HG)
    NXT = len(xt_chunks)
    NHG = len(h_groups)
    chunk_of_tile = []
    for r, (t0, tn) in enumerate(xt_chunks):
        chunk_of_tile += [r] * tn

    NW = len(wins)
    # retire count of the phase-B matmul that consumes global chunk j
    mm_retire = []
    mm_cum = [0]                       # matmuls through slot s (incl. bias)
    for s in range(NB):
        base = mm_cum[-1] + 1          # bias matmul of slot s
        for c in range(K[s]):
            mm_retire.append(base + c + 1)
        mm_cum.append(base + K[s])
    win_of_chunk = []
    for w, (j0, jn) in enumerate(wins):
        win_of_chunk += [w] * jn

    from contextlib import ExitStack

    with ExitStack() as es:
        ph = [es.enter_context(nc.psum_tensor(f"ph{k}", [128, 512], FP32))
              for k in range(PSA)]
        pbk = [es.enter_context(nc.psum_tensor(f"pb{k}", [128, 512], FP32))
               for k in range(2)]
        w_sb = es.enter_context(nc.sbuf_tensor("w_sb", [128, 2, 128], FP16))
        on_sb = es.enter_context(nc.sbuf_tensor("on_sb", [1, 128], FP16))
        b_sb = es.enter_context(nc.sbuf_tensor("b_sb", [1, 128], FP16))
        iota_sb = es.enter_context(nc.sbuf_tensor("iota_sb", [128, 128], FP16))
        col_sb = es.enter_context(nc.sbuf_tensor("col_sb", [128, cidx], I16))
        rl_sb = es.enter_context(nc.sbuf_tensor("rl_sb", [128, KTOT], FP32))
        xt_sb = es.enter_context(
            nc.sbuf_tensor("xt_sb", [128, 2, XT_CH, 2, 128], FP16)
        )
        h_sb = es.enter_context(
            nc.sbuf_tensor("h_sb", [128, HRING * HG, 128], FP16)
        )
        val_sb = es.enter_context(
            nc.sbuf_tensor("val_sb", [128, VR, 128], FP16)
        )
        s_sb = es.enter_context(nc.sbuf_tensor("s_sb", [128, SB, 128], FP16))
        o_sb = es.enter_context(nc.sbuf_tensor("o_sb", [128, 2, 128], FP32))

        s_ld = es.enter_context(nc.semaphore("s_ld"))       # one-time loads
        s_ldw = es.enter_context(nc.semaphore("s_ldw"))     # weight loads
        s_xt = [es.enter_context(nc.semaphore(f"s_xt{k}")) for k in range(2)]
        s_hw = [
            es.enter_context(nc.semaphore(f"s_hw{k}")) for k in range(HRING)
        ]
        s_hwx = es.enter_context(nc.semaphore("s_hwx"))     # odd-tile stores
        s_gat = [
            es.enter_context(nc.semaphore(f"s_gat{k}")) for k in range(NGS)
        ]
        s_ow = [es.enter_context(nc.semaphore(f"s_ow{k}")) for k in range(2)]
        s_hmm = es.enter_context(nc.semaphore("s_hmm"))
        s_hcp = es.enter_context(nc.semaphore("s_hcp"))
        s_s = es.enter_context(nc.semaphore("s_s"))
        s_pmm = es.enter_context(nc.semaphore("s_pmm"))
        s_ocp = es.enter_context(nc.semaphore("s_ocp"))
        block = es.enter_context(nc.Block())

        hw_total = [16 * len(range(k, NHG, HRING)) for k in range(HRING)]
        n_odd = sum(1 for g0, gn in h_groups if gn % 2)

        def store_h(sync, ns):
            """Store h group ns with 512B descriptors: DRAM rows are
            pair-interleaved (u = t2*256 + 2p + half) so two tiles' rows for
            one partition are adjacent; an odd trailing tile stores alone."""
            g0, gn = h_groups[ns]
            a = (ns % HRING) * HG
            sync.wait_ge(s_hcp, g0 + gn)
            gp = gn - (gn % 2)
            sync.dma_start(
                h_d[g0 * 128:(g0 + gp) * 128, :].rearrange(
                    "(t2 p two) f -> p t2 (two f)", p=128, two=2
                ),
                h_sb[:, a:a + gp, :].rearrange(
                    "p (t2 two) f -> p t2 (two f)", two=2
                ),
            ).then_inc(s_hw[ns % HRING], 16)
            if gn % 2:
                r0 = (g0 + gp) * 128
                # write the lone tile twice (even + odd half-rows) so the
                # pair's unwritten half stays finite for the gather's checks
                for half in range(2):
                    sync.dma_start(
                        h_d[r0:r0 + 256, :].rearrange(
                            "(p two) f -> p two f", two=2
                        )[:, half, :],
                        h_sb[:, a + gp, :],
                    ).then_inc(s_hwx, 16)

        @block.sync
        def _(sync):
            # phase A first: the PE only needs xt chunk 0 + W to start, so
            # those go ahead of the bulky phase-B tables (col is 1.3 MB)
            ns = 0
            for r, (t0, tn) in enumerate(xt_chunks):
                if r == 1:
                    sync.dma_start(w_sb[:, 0, :], w_d[0]).then_inc(s_ldw, 16)
                    sync.dma_start(w_sb[:, 1, :], w_d[1]).then_inc(s_ldw, 16)
                elif r == 2:
                    sync.dma_start(iota_sb[:, :], io_d[:, :]).then_inc(
                        s_ld, 16
                    )
                    sync.dma_start(rl_sb[:, :], rl_d[:, :]).then_inc(s_ld, 16)
                    sync.dma_start(col_sb[:, :], col_d[:, :]).then_inc(
                        s_ld, 16
                    )
                    sync.dma_start(on_sb[:, :], on_d[:, :]).then_inc(s_ld, 16)
                    sync.dma_start(b_sb[:, :], b_d[:, :]).then_inc(s_ld, 16)
                if r >= 2:
                    pt0, ptn = xt_chunks[r - 2]
                    sync.wait_ge(s_hmm, pt0 + ptn)
                sync.dma_start(
                    xt_sb[:, r % 2, 0:tn, :, :].opt(),
                    xt_d[:, t0 * 256:(t0 + tn) * 256],
                ).then_inc(s_xt[r % 2], 16)
                while ns < NHG and h_groups[ns][0] + h_groups[ns][1] <= t0:
                    store_h(sync, ns)
                    ns += 1
            while ns < NHG:
                store_h(sync, ns)
                ns += 1
            # phase B: output stores
            for s in range(NB):
                sync.wait_ge(s_ocp, s + 1)
                sync.dma_start(
                    o_d[s * 128:(s + 1) * 128, :], o_sb[:, s % 2, :]
                ).then_inc(s_ow[s % 2], 16)

        @block.gpsimd
        def _(gpsimd):
            gpsimd.wait_ge(s_ld, 16 * 5)
            for k in range(HRING):
                gpsimd.wait_ge(s_hw[k], hw_total[k])
            gpsimd.wait_ge(s_hwx, 32 * n_odd)
            for w, (j0, jn) in enumerate(wins):
                j1 = j0 + jn
                if j1 > VR:
                    gpsimd.wait_ge(s_pmm, mm_retire[j1 - VR - 1])
                gpsimd.dma_gather(
                    val_sb[:, j0 % VR:j0 % VR + jn, :],
                    h_d[:, :],
                    col_sb[:, j0 * 8:j1 * 8],
                    jn * 128,
                    jn * 128,
                    128,
                ).then_inc(s_gat[w % NGS], 16)

        @block.tensor
        def _(tensor):
            tensor.wait_ge(s_ldw, 32)
            # phase A: h tile i = xt_i^T @ W  (two K chunks)
            for i in range(NT):
                r = chunk_of_tile[i]
                if i == xt_chunks[r][0]:
                    tensor.wait_ge(s_xt[r % 2], 16 * (r // 2 + 1))
                if i >= PSA:
                    tensor.wait_ge(s_hcp, i - (PSA - 1))
                tl = i - xt_chunks[r][0]
                tensor.matmul(
                    ph[i % PSA][:, 0:128],
                    xt_sb[:, r % 2, tl, 0, :],
                    w_sb[:, 0, :],
                    start=True,
                    stop=False,
                )
                tensor.matmul(
                    ph[i % PSA][:, 0:128],
                    xt_sb[:, r % 2, tl, 1, :],
                    w_sb[:, 1, :],
                    start=False,
                    stop=True,
                ).then_inc(s_hmm, 1)
            # phase B: out_slot = ones^T @ bias + sum_c S_c^T @ val_c
            tensor.wait_ge(s_ld, 16 * 5)
            j = 0
            for s in range(NB):
                if s >= 2:
                    tensor.wait_ge(s_ocp, s - 1)
                tensor.matmul(
                    pbk[s % 2][:, 0:128],
                    on_sb[:, :],
                    b_sb[:, :],
                    start=True,
                    stop=False,
                ).then_inc(s_pmm, 1)
                for c in range(K[s]):
                    tensor.wait_ge(s_s, j + 1)
                    tensor.matmul(
                        pbk[s % 2][:, 0:128],
                        s_sb[:, j % SB, :],
                        val_sb[:, j % VR, :],
                        start=False,
                        stop=(c == K[s] - 1),
                    ).then_inc(s_pmm, 1)
                    j += 1

        @block.vector
        def _(vector):
            # phase A: PSUM fp32 -> SBUF fp16
            for i in range(NT):
                vector.wait_ge(s_hmm, i + 1)
                g = i // HG
                if g >= HRING and i == g * HG:
                    vector.wait_ge(s_hw[g % HRING], 16 * (g // HRING))
                vector.tensor_copy(
                    h_sb[:, (g % HRING) * HG + (i - g * HG), :],
                    ph[i % PSA][:, 0:128],
                ).then_inc(s_hcp, 1)
            # phase B: one-hot tiles S[e, n] = (iota[n] == rowloc[e])
            for j in range(KTOT):
                w = win_of_chunk[j]
                if j == 0 or win_of_chunk[j - 1] != w:
                    vector.wait_ge(s_gat[w % NGS], 16 * (w // NGS + 1))
                if j >= SB:
                    vector.wait_ge(s_pmm, mm_retire[j - SB])
                vector.tensor_scalar(
                    s_sb[:, j % SB, :],
                    iota_sb[:, :],
                    rl_sb[:, j:j + 1],
                    None,
                    mybir.AluOpType.is_equal,
                ).then_inc(s_s, 1)

        @block.scalar
        def _(scalar):
            for s in range(NB):
                scalar.wait_ge(s_pmm, mm_cum[s + 1])
                if s >= 2:
                    scalar.wait_ge(s_ow[s % 2], 16 * (s // 2))
                scalar.activation(
                    o_sb[:, s % 2, :],
                    pbk[s % 2][:, 0:128],
                    mybir.ActivationFunctionType.Relu,
                ).then_inc(s_ocp, 1)

    nc.compile()
    return nc


def _run(x, edge_index, weight, bias, trace=False):
    shared, per_core, plan = _host_prep(x, edge_index, weight, bias)
    nc = _build_program(plan)
    in_maps = [dict(shared, **per_core[c]) for c in range(NCORES)]
    res = run_bass_kernel_spmd(nc, in_maps, list(range(NCORES)), trace=trace)
    out = np.zeros((NPAD, FOUT), np.float32)
    for c in range(NCORES):
        oc = res.results[c]["out"]
        for s in range(NB):
            b = plan["block_of"][c][s]
            if b is not None:
                out[b * 128:(b + 1) * 128] = oc[s * 128:(s + 1) * 128]
    return np.ascontiguousarray(out[:N_NODES]), res


def kernel(x, edge_index, weight, bias):
    out, _ = _run(x, edge_index, weight, bias, trace=False)
    return out


# revision 37
# speedup vs baseline: 1.1400x; 1.0321x over previous
"""GNN message-passing (graph convolution) kernel for 8 Trainium2 NeuronCores.

    out = relu(segment_sum(h[col], row) + bias),  h = x @ W

Strategy (dst-block sharding -- no collectives needed):
  * Host sorts edges by destination node, buckets them into 157 blocks of 128
    dst nodes, sorts blocks by edge count and deals them snake-style into 20
    slots x 8 cores so every core runs the same instruction stream with
    near-balanced per-slot chunk budgets.  Each core produces a disjoint slice
    of the output, so no all-reduce is needed.
  * Phase A (per core, replicated): h = x @ W on the PE in fp16 (PSUM fp32
    accumulate).  x is shipped pre-transposed in a partition-major layout so
    the whole 10 MB streams in as 5 large DMAs (16 KB contiguous per
    partition per DMA -- full DMA-bus rate, HWDGE amortized).  h tiles are
    copied PSUM->SBUF on the DVE and stored to a DRAM buffer h[20096,128]
    fp16 in 16-tile batched DMAs.
  * Phase B: per dst slot, SWDGE dma_gather fetches the slot's (padded) edge
    h rows into SBUF [128e, K, 128f] in sub-gathers of 16 chunks (2048
    descriptors; the SWDGE ring is enlarged via dynamic_dma_scratch_size);
    the DVE builds one-hot tiles S[e,n] = (iota == rowloc) in fp16 (waiting
    on the gather so the PE needs only one wait per matmul); the PE
    accumulates out_block = ones^T @ bias + sum_c S_c^T @ val_c in PSUM fp32
    -- an exact segment-sum with the bias folded in as a K=1 matmul.  ACT
    applies ReLU PSUM->SBUF and results stream out per slot.

Numerics: fp16 operands with fp32 accumulation everywhere; the one-hot matmul
is exact, so the only error is fp16 rounding of x, W and h (~1e-3 relative).
"""

import sys

import numpy as np

sys.path.insert(0, "/opt/trn_rl_repo")

import concourse.bacc as bacc  # noqa: E402
import concourse.bass as bass  # noqa: E402  (engine types)
import concourse.mybir as mybir  # noqa: E402
from concourse.bass_utils import run_bass_kernel_spmd  # noqa: E402

N_NODES = 20000
FIN = 256
FOUT = 128
N_EDGES = 640000

NT = 157                 # node tiles of 128 (nodes padded to 20096)
NPAD = NT * 128          # 20096
NBLK = 157               # dst blocks of 128 nodes
NCORES = 8
NB = 20                  # block slots per core

XT_CH = 20               # xt tiles per load DMA (8 DMAs: 20*7 + 17)
XTR = 4                  # xt chunk ring depth
HG = 16                  # h tiles per store DMA (10 DMAs: 16*9 + 13)
SUBG = 8                 # chunks per sub-gather (1024 descriptors)
PSA = 4                  # phase-A psum ring
SB = 6                   # one-hot tile ring
HRING = 3                # h store group ring depth
VR = 64                  # val chunk ring (multiple of SUBG)
NGS = 8                  # gather completion sem rotation

FP16 = mybir.dt.float16
FP32 = mybir.dt.float32
I16 = mybir.dt.int16


def _chunks(total, step):
    out = []
    o = 0
    while o < total:
        out.append((o, min(step, total - o)))
        o += step
    return out


def _host_prep(x, edge_index, weight, bias):
    """Cast/retile operands, bucket + balance edges by destination block.

    Returns (shared, per_core, plan): shared tensors (same on every core),
    per-core col/rl tensors, and the static plan consumed by _build_program.
    """
    x = np.asarray(x, np.float32)
    weight = np.asarray(weight, np.float32)
    bias = np.asarray(bias, np.float32)

    xpad = np.zeros((NPAD, FIN), np.float32)
    xpad[:N_NODES] = x
    # partition-major lhsT: xt[k, i, kc, m] = x[i*128+m, kc*128+k]
    xt = np.ascontiguousarray(
        xpad.reshape(NT, 128, 2, 128).transpose(3, 0, 2, 1).astype(np.float16)
    ).reshape(128, NT * 2 * 128)
    w_t = np.ascontiguousarray(weight.astype(np.float16).reshape(2, 128, 128))
    ones16 = np.ones((1, 128), np.float16)
    bias16 = np.ascontiguousarray(bias.astype(np.float16).reshape(1, 128))
    iota16 = np.ascontiguousarray(
        np.broadcast_to(np.arange(128, dtype=np.float16), (128, 128))
    )

    row = np.asarray(edge_index[0]).astype(np.int64)
    col = np.asarray(edge_index[1]).astype(np.int64)
    order = np.argsort(row, kind="stable")
    rs = row[order].astype(np.int32)
    cs = col[order].astype(np.int32)

    blk = rs >> 7
    counts = np.bincount(blk, minlength=NBLK)
    starts = np.concatenate([[0], np.cumsum(counts)])

    # deal blocks (sorted by descending count) into slots: slot s holds the
    # 8 blocks ranked [8s, 8s+8); per-slot chunk budget = max count in group
    rank = np.argsort(-counts, kind="stable")
    block_of = [[None] * NB for _ in range(NCORES)]
    K = [0] * NB                      # chunks per slot
    n_ex = [0] * NB                   # exact (16-rounded) idxs per slot
    for s in range(NB):
        grp = rank[8 * s:8 * s + 8]
        for c, b in enumerate(grp):
            block_of[c][s] = int(b)
        n_ex[s] = max(16, -(-int(counts[grp].max()) // 16) * 16)
        K[s] = -(-n_ex[s] // 128)
    KTOT = sum(K)
    # flat chunk stream: slot boundaries only matter to the matmul schedule;
    # gathers run in cross-slot windows of SUBG full chunks
    wins = _chunks(KTOT, SUBG)                            # (chunk_off, len)

    # per-core index/rowloc buffers (idx stream wrapped per window)
    cidx = 8 * KTOT                    # int16 per partition
    col16 = np.zeros((NCORES, 128, cidx), np.int16)
    rl16 = np.full((NCORES, 128, KTOT), -1.0, np.float32)
    tile_need = np.zeros(KTOT, np.int64)   # max h tile per chunk (over cores)
    for core in range(NCORES):
        lin_col = np.zeros(KTOT * 128, np.int32)
        ko = 0
        for s in range(NB):
            n_pad = K[s] * 128
            lin_rl = np.full(n_pad, -1.0, np.float32)
            b = block_of[core][s]
            if b is not None:
                e0, e1 = int(starts[b]), int(starts[b + 1])
                k = e1 - e0
                # sort by col so early chunks only need early h tiles
                so = np.argsort(cs[e0:e1], kind="stable")
                cse = cs[e0:e1][so]
                # pair-interleaved h row: u = t2*256 + 2p + half
                lin_col[ko * 128:ko * 128 + k] = (
                    (cse >> 8 << 8) + ((cse & 127) << 1) + ((cse >> 7) & 1)
                )
                lin_rl[:k] = rs[e0:e1][so] - b * 128
                ctile = np.zeros(K[s] * 128, np.int64)
                ctile[:k] = cse >> 7
                tile_need[ko:ko + K[s]] = np.maximum(
                    tile_need[ko:ko + K[s]],
                    np.maximum.reduceat(ctile, np.arange(0, K[s] * 128, 128)),
                )
            rl16[core, :, ko:ko + K[s]] = (
                lin_rl.reshape(K[s], 128).T.astype(np.float32)
            )
            ko += K[s]
        assert ko == KTOT
        for j0, jn in wins:
            L = jn * 128
            piece = lin_col[j0 * 128:j0 * 128 + L]
            col16[core, :, j0 * 8:j0 * 8 + L // 16] = np.tile(
                piece.reshape(L // 16, 16).T.astype(np.int16), (8, 1)
            )

    shared = {
        "xt": xt,
        "w": w_t,
        "ones": ones16,
        "bias": bias16,
        "iota": iota16,
    }
    per_core = [
        {"col": np.ascontiguousarray(col16[c]),
         "rl": np.ascontiguousarray(rl16[c])}
        for c in range(NCORES)
    ]
    win_need = [int(tile_need[j0:j0 + jn].max()) for j0, jn in wins]
    plan = {"K": K, "KTOT": KTOT, "wins": wins, "cidx": cidx,
            "block_of": block_of, "win_need": win_need}
    return shared, per_core, plan


def _build_program(plan):
    K, KTOT, wins, cidx, win_need = (
        plan["K"], plan["KTOT"], plan["wins"], plan["cidx"], plan["win_need"]
    )
    nc = bacc.Bacc("TRN2", dynamic_dma_scratch_size=16384)

    xt_d = nc.dram_tensor("xt", [128, NT * 2 * 128], FP16, kind="ExternalInput")
    w_d = nc.dram_tensor("w", [2, 128, 128], FP16, kind="ExternalInput")
    on_d = nc.dram_tensor("ones", [1, 128], FP16, kind="ExternalInput")
    b_d = nc.dram_tensor("bias", [1, 128], FP16, kind="ExternalInput")
    io_d = nc.dram_tensor("iota", [128, 128], FP16, kind="ExternalInput")
    col_d = nc.dram_tensor("col", [128, cidx], I16, kind="ExternalInput")
    rl_d = nc.dram_tensor("rl", [128, KTOT], FP32, kind="ExternalInput")
    h_d = nc.dram_tensor("hbuf", [(NT + 1) // 2 * 256, 128], FP16)
    o_d = nc.dram_tensor("out", [NB * 128, 128], FP32, kind="ExternalOutput")

    xt_chunks = _chunks(NT, XT_CH)     # (tile_off, n_tiles)
    h_groups = _chunks(NT, HG)
    NXT = len(xt_chunks)
    NHG = len(h_groups)
    chunk_of_tile = []
    for r, (t0, tn) in enumerate(xt_chunks):
        chunk_of_tile += [r] * tn

    NW = len(wins)
    # retire count of the phase-B matmul that consumes global chunk j
    mm_retire = []
    mm_cum = [0]                       # matmuls through slot s (incl. bias)
    for s in range(NB):
        base = mm_cum[-1] + 1          # bias matmul of slot s
        for c in range(K[s]):
            mm_retire.append(base + c + 1)
        mm_cum.append(base + K[s])
    win_of_chunk = []
    for w, (j0, jn) in enumerate(wins):
        win_of_chunk += [w] * jn

    from contextlib import ExitStack

    with ExitStack() as es:
        ph = [es.enter_context(nc.psum_tensor(f"ph{k}", [128, 512], FP32))
              for k in range(PSA)]
        pbk = [es.enter_context(nc.psum_tensor(f"pb{k}", [128, 512], FP32))
               for k in range(2)]
        w_sb = es.enter_context(nc.sbuf_tensor("w_sb", [128, 2, 128], FP16))
        on_sb = es.enter_context(nc.sbuf_tensor("on_sb", [1, 128], FP16))
        b_sb = es.enter_context(nc.sbuf_tensor("b_sb", [1, 128], FP16))
        iota_sb = es.enter_context(nc.sbuf_tensor("iota_sb", [128, 128], FP16))
        col_sb = es.enter_context(nc.sbuf_tensor("col_sb", [128, cidx], I16))
        rl_sb = es.enter_context(nc.sbuf_tensor("rl_sb", [128, KTOT], FP32))
        xt_sb = es.enter_context(
            nc.sbuf_tensor("xt_sb", [128, XTR, XT_CH, 2, 128], FP16)
        )
        h_sb = es.enter_context(
            nc.sbuf_tensor("h_sb", [128, HRING * HG, 128], FP16)
        )
        val_sb = es.enter_context(
            nc.sbuf_tensor("val_sb", [128, VR, 128], FP16)
        )
        s_sb = es.enter_context(nc.sbuf_tensor("s_sb", [128, SB, 128], FP16))
        o_sb = es.enter_context(nc.sbuf_tensor("o_sb", [128, 2, 128], FP32))

        s_ld = es.enter_context(nc.semaphore("s_ld"))       # one-time loads
        s_ldw = es.enter_context(nc.semaphore("s_ldw"))     # weight loads
        s_xt = [
            es.enter_context(nc.semaphore(f"s_xt{k}")) for k in range(XTR)
        ]
        s_hw = [
            es.enter_context(nc.semaphore(f"s_hw{k}")) for k in range(HRING)
        ]
        s_hwx = es.enter_context(nc.semaphore("s_hwx"))     # odd-tile stores
        s_gat = [
            es.enter_context(nc.semaphore(f"s_gat{k}")) for k in range(NGS)
        ]
        s_ow = [es.enter_context(nc.semaphore(f"s_ow{k}")) for k in range(2)]
        s_hmm = es.enter_context(nc.semaphore("s_hmm"))
        s_hcp = es.enter_context(nc.semaphore("s_hcp"))
        s_s = es.enter_context(nc.semaphore("s_s"))
        s_pmm = es.enter_context(nc.semaphore("s_pmm"))
        s_ocp = es.enter_context(nc.semaphore("s_ocp"))
        block = es.enter_context(nc.Block())

        hw_total = [16 * len(range(k, NHG, HRING)) for k in range(HRING)]
        n_odd = sum(1 for g0, gn in h_groups if gn % 2)

        def store_h(sync, ns):
            """Store h group ns with 512B descriptors: DRAM rows are
            pair-interleaved (u = t2*256 + 2p + half) so two tiles' rows for
            one partition are adjacent; an odd trailing tile stores alone."""
            g0, gn = h_groups[ns]
            a = (ns % HRING) * HG
            sync.wait_ge(s_hcp, g0 + gn)
            gp = gn - (gn % 2)
            sync.dma_start(
                h_d[g0 * 128:(g0 + gp) * 128, :].rearrange(
                    "(t2 p two) f -> p t2 (two f)", p=128, two=2
                ),
                h_sb[:, a:a + gp, :].rearrange(
                    "p (t2 two) f -> p t2 (two f)", two=2
                ),
            ).then_inc(s_hw[ns % HRING], 16)
            if gn % 2:
                r0 = (g0 + gp) * 128
                # write the lone tile twice (even + odd half-rows) so the
                # pair's unwritten half stays finite for the gather's checks
                for half in range(2):
                    sync.dma_start(
                        h_d[r0:r0 + 256, :].rearrange(
                            "(p two) f -> p two f", two=2
                        )[:, half, :],
                        h_sb[:, a + gp, :],
                    ).then_inc(s_hwx, 16)

        @block.sync
        def _(sync):
            # phase A first: the PE only needs xt chunk 0 + W to start, so
            # those go ahead of the bulky phase-B tables (col is 1.3 MB).
            # h stores are issued from the (otherwise idle) ACT queue so
            # their s_hcp waits never stall the xt stream here.
            for r, (t0, tn) in enumerate(xt_chunks):
                if r == 1:
                    sync.dma_start(w_sb[:, 0, :], w_d[0]).then_inc(s_ldw, 16)
                    sync.dma_start(w_sb[:, 1, :], w_d[1]).then_inc(s_ldw, 16)
                elif r == 2:
                    sync.dma_start(iota_sb[:, :], io_d[:, :]).then_inc(
                        s_ld, 16
                    )
                    sync.dma_start(rl_sb[:, :], rl_d[:, :]).then_inc(s_ld, 16)
                    sync.dma_start(col_sb[:, :], col_d[:, :]).then_inc(
                        s_ld, 16
                    )
                    sync.dma_start(on_sb[:, :], on_d[:, :]).then_inc(s_ld, 16)
                    sync.dma_start(b_sb[:, :], b_d[:, :]).then_inc(s_ld, 16)
                if r >= XTR:
                    pt0, ptn = xt_chunks[r - XTR]
                    sync.wait_ge(s_hmm, pt0 + ptn)
                sync.dma_start(
                    xt_sb[:, r % XTR, 0:tn, :, :].opt(),
                    xt_d[:, t0 * 256:(t0 + tn) * 256],
                ).then_inc(s_xt[r % XTR], 16)
            # phase B: output stores
            for s in range(NB):
                sync.wait_ge(s_ocp, s + 1)
                sync.dma_start(
                    o_d[s * 128:(s + 1) * 128, :], o_sb[:, s % 2, :]
                ).then_inc(s_ow[s % 2], 16)

        @block.gpsimd
        def _(gpsimd):
            gpsimd.wait_ge(s_ld, 16 * 5)
            # store-progress waits: window w only needs h tiles <= win_need[w]
            # (slots are col-sorted), so early windows overlap phase A's tail
            hw_seen = [0] * HRING
            hwx_seen = 0
            for w, (j0, jn) in enumerate(wins):
                gw = win_need[w] // HG     # last h store group this window needs
                need = [0] * HRING
                needx = 0
                for g in range(gw + 1):
                    need[g % HRING] += 16
                    if h_groups[g][1] % 2:
                        needx += 32
                for k in range(HRING):
                    if need[k] > hw_seen[k]:
                        gpsimd.wait_ge(s_hw[k], need[k])
                        hw_seen[k] = need[k]
                if needx > hwx_seen:
                    gpsimd.wait_ge(s_hwx, needx)
                    hwx_seen = needx
                j1 = j0 + jn
                if j1 > VR:
                    gpsimd.wait_ge(s_pmm, mm_retire[j1 - VR - 1])
                gpsimd.dma_gather(
                    val_sb[:, j0 % VR:j0 % VR + jn, :],
                    h_d[:, :],
                    col_sb[:, j0 * 8:j1 * 8],
                    jn * 128,
                    jn * 128,
                    128,
                ).then_inc(s_gat[w % NGS], 16)

        @block.tensor
        def _(tensor):
            tensor.wait_ge(s_ldw, 32)
            # phase A: h tile i = xt_i^T @ W  (two K chunks)
            for i in range(NT):
                r = chunk_of_tile[i]
                if i == xt_chunks[r][0]:
                    tensor.wait_ge(s_xt[r % XTR], 16 * (r // XTR + 1))
                if i >= PSA:
                    tensor.wait_ge(s_hcp, i - (PSA - 1))
                tl = i - xt_chunks[r][0]
                tensor.matmul(
                    ph[i % PSA][:, 0:128],
                    xt_sb[:, r % XTR, tl, 0, :],
                    w_sb[:, 0, :],
                    start=True,
                    stop=False,
                )
                tensor.matmul(
                    ph[i % PSA][:, 0:128],
                    xt_sb[:, r % XTR, tl, 1, :],
                    w_sb[:, 1, :],
                    start=False,
                    stop=True,
                ).then_inc(s_hmm, 1)
            # phase B: out_slot = ones^T @ bias + sum_c S_c^T @ val_c
            tensor.wait_ge(s_ld, 16 * 5)
            j = 0
            for s in range(NB):
                if s >= 2:
                    tensor.wait_ge(s_ocp, s - 1)
                tensor.matmul(
                    pbk[s % 2][:, 0:128],
                    on_sb[:, :],
                    b_sb[:, :],
                    start=True,
                    stop=False,
                ).then_inc(s_pmm, 1)
                for c in range(K[s]):
                    tensor.wait_ge(s_s, j + 1)
                    tensor.matmul(
                        pbk[s % 2][:, 0:128],
                        s_sb[:, j % SB, :],
                        val_sb[:, j % VR, :],
                        start=False,
                        stop=(c == K[s] - 1),
                    ).then_inc(s_pmm, 1)
                    j += 1

        @block.vector
        def _(vector):
            # phase A: PSUM fp32 -> SBUF fp16
            for i in range(NT):
                vector.wait_ge(s_hmm, i + 1)
                g = i // HG
                if g >= HRING and i == g * HG:
                    vector.wait_ge(s_hw[g % HRING], 16 * (g // HRING))
                vector.tensor_copy(
                    h_sb[:, (g % HRING) * HG + (i - g * HG), :],
                    ph[i % PSA][:, 0:128],
                ).then_inc(s_hcp, 1)
            # phase B: one-hot tiles S[e, n] = (iota[n] == rowloc[e])
            for j in range(KTOT):
                w = win_of_chunk[j]
                if j == 0 or win_of_chunk[j - 1] != w:
                    vector.wait_ge(s_gat[w % NGS], 16 * (w // NGS + 1))
                if j >= SB:
                    vector.wait_ge(s_pmm, mm_retire[j - SB])
                vector.tensor_scalar(
                    s_sb[:, j % SB, :],
                    iota_sb[:, :],
                    rl_sb[:, j:j + 1],
                    None,
                    mybir.AluOpType.is_equal,
                ).then_inc(s_s, 1)

        @block.scalar
        def _(scalar):
            # phase A: h group stores (paced by copy progress, not SP)
            for ns in range(NHG):
                store_h(scalar, ns)
            # phase B: ReLU PSUM -> SBUF per slot
            for s in range(NB):
                scalar.wait_ge(s_pmm, mm_cum[s + 1])
                if s >= 2:
                    scalar.wait_ge(s_ow[s % 2], 16 * (s // 2))
                scalar.activation(
                    o_sb[:, s % 2, :],
                    pbk[s % 2][:, 0:128],
                    mybir.ActivationFunctionType.Relu,
                ).then_inc(s_ocp, 1)

    nc.compile()
    return nc


def _run(x, edge_index, weight, bias, trace=False):
    shared, per_core, plan = _host_prep(x, edge_index, weight, bias)
    nc = _build_program(plan)
    in_maps = [dict(shared, **per_core[c]) for c in range(NCORES)]
    res = run_bass_kernel_spmd(nc, in_maps, list(range(NCORES)), trace=trace)
    out = np.zeros((NPAD, FOUT), np.float32)
    for c in range(NCORES):
        oc = res.results[c]["out"]
        for s in range(NB):
            b = plan["block_of"][c][s]
            if b is not None:
                out[b * 128:(b + 1) * 128] = oc[s * 128:(s + 1) * 128]
    return np.ascontiguousarray(out[:N_NODES]), res


def kernel(x, edge_index, weight, bias):
    out, _ = _run(x, edge_index, weight, bias, trace=False)
    return out
